# revision 1
# baseline (speedup 1.0000x reference)
"""BipartiteSAGEConv on 8 Trainium2 NeuronCores.

out = normalize(mean_{dst}(x[src]) @ W_l + b_l + x @ W_r)

Strategy:
- Host: sort edges by destination node, shard destination-node ranges across
  the 8 cores (each core owns 12500 contiguous nodes and all edges pointing
  into them -> no cross-core reduction needed). Per 128-node tile, edges are
  grouped by src bank (4 banks of 25024 rows, since dma_gather indices are
  int16) and packed into KB chunks of 128 per bank (padded; padding edges
  carry dstrel=-1 so the one-hot kills them). The per-edge weight
  w = 1/max(deg(dst),1) is folded into the one-hot so the PSUM accumulation
  yields the mean directly.
- Device (SPMD, identical program on all 8 cores):
  * dma_gather (Ant SWDGE gather, int16 idx) of x[src] rows per (tile, bank)
  * DVE builds the weighted one-hot: (iota == dstrel) * w  (one fused op)
  * PE accumulates meanT[f, n] += msg[e, f].T @ onehot[e, n] over chunks
  * PE: out[n, fo] = meanT.T @ W_l + xT.T @ W_r + ones x b_l (one PSUM group)
  * ACT Square+accum -> row sum of squares; sqrt; clamp; DVE reciprocal;
    scale rows; DMA out.
"""

import numpy as np

N_NODES = 100000
D = 128
N_CORES = 8
NODES_PER_CORE = N_NODES // N_CORES  # 12500
P = 128
TILES_PER_CORE = (NODES_PER_CORE + P - 1) // P  # 98
NODE_PAD = TILES_PER_CORE * P  # 12544
X_PAD_ROWS = 100096  # 782 * 128; >= 7*12500 + 12544
BANK = X_PAD_ROWS // 4  # 25024 rows per gather bank (< 32768 int16 limit)
NBANKS = 4

_program_cache = {}

# test harness hooks
TRACE = False
LAST = {}

NQUEUES = 4  # SWDGE queues; gathers round-robin across them
SCRATCH = 16384  # SWDGE descriptor-ring carveout bytes (ring = SCRATCH // 16)
GATHER_BF16 = True  # gather from a bf16 copy of x (halves gather bytes);
SINGLE_PACKET = True  # dma_gather packet mode (HW ucode knob, A/B on HW)
# the one-hot is then exact 0/1 bf16 and the 1/deg scaling happens in f32
# on the (summed @ W_l) product instead of being folded into the one-hot.


def _build_program(KB: int, bench_repeat: int = 1, ablate: str = ""):
    """Build + compile the SPMD Bass program; KB = edge chunks per (tile, bank).

    bench_repeat > 1 wraps the main loop in a For_i that recomputes the same
    output bench_repeat times (for device-time measurement only).
    ablate: comma-set of {gather, onehot, chunkmm} to skip (bench only).
    """
    ablate_set = set(ablate.split(",")) if ablate else set()
    import contextlib

    import concourse.bass as bass
    import concourse.tile as tile
    from concourse import bacc, mybir
    from concourse.masks import make_identity

    f32 = mybir.dt.float32
    bf16 = mybir.dt.bfloat16
    i16 = mybir.dt.int16
    gdt = bf16 if GATHER_BF16 else f32
    KT = NBANKS * KB  # chunk slots per tile
    NIDX = KB * P  # indices per gather
    IW = NIDX // 16  # idx columns per (tile, bank)

    nc = bacc.Bacc(
        "TRN2",
        target_bir_lowering=False,
        debug=False,
        num_devices=N_CORES,
        num_swdge_queues=NQUEUES,
        dynamic_dma_scratch_size=SCRATCH,
    )

    if GATHER_BF16:
        xgat = nc.dram_tensor("xbf", [X_PAD_ROWS, D], bf16, kind="ExternalInput")
        rcol = nc.dram_tensor("rcol", [P, TILES_PER_CORE], f32, kind="ExternalInput")
    else:
        xgat = nc.dram_tensor("xpad", [X_PAD_ROWS, D], f32, kind="ExternalInput")
    xchunk = nc.dram_tensor("xchunk", [NODE_PAD, D], f32, kind="ExternalInput")
    gidx = nc.dram_tensor(
        "gidx", [P, TILES_PER_CORE, NBANKS, IW], i16, kind="ExternalInput"
    )
    dstrel = nc.dram_tensor("dstrel", [P, TILES_PER_CORE, KT], f32, kind="ExternalInput")
    wgt = nc.dram_tensor("wgt", [P, TILES_PER_CORE, KT], f32, kind="ExternalInput")
    wl = nc.dram_tensor("wl", [D, D], f32, kind="ExternalInput")
    wr = nc.dram_tensor("wr", [D, D], f32, kind="ExternalInput")
    bl = nc.dram_tensor("bl", [1, D], f32, kind="ExternalInput")
    out = nc.dram_tensor("out", [NODE_PAD, D], f32, kind="ExternalOutput")

    with tile.TileContext(nc) as tc:
        with (
            tc.tile_pool(name="const", bufs=1) as const_pool,
            tc.tile_pool(name="meta", bufs=1) as meta_pool,
            tc.tile_pool(name="msg", bufs=3 * NBANKS) as msg_pool,
            tc.tile_pool(name="oh", bufs=6) as oh_pool,
            tc.tile_pool(name="xt", bufs=3) as xt_pool,
            tc.tile_pool(name="ep", bufs=3) as ep_pool,
            tc.tile_pool(name="ps_mean", bufs=2, space="PSUM") as ps_mean_pool,
            tc.tile_pool(name="ps_xt", bufs=2, space="PSUM") as ps_xt_pool,
            tc.tile_pool(name="ps_out", bufs=2, space="PSUM") as ps_out_pool,
        ):
            # ---- constants / weights / metadata (loaded once) ----
            iota_i = const_pool.tile([P, P], mybir.dt.int32)
            nc.gpsimd.iota(iota_i[:], pattern=[[1, P]], base=0, channel_multiplier=0)
            iota_f = const_pool.tile([P, P], f32)
            nc.vector.tensor_copy(iota_f[:], iota_i[:])
            if GATHER_BF16:
                iota_g = const_pool.tile([P, P], bf16)
                nc.vector.tensor_copy(iota_g[:], iota_i[:])
                rcol_sb = const_pool.tile([P, TILES_PER_CORE], f32)
                nc.sync.dma_start(rcol_sb[:], rcol[:])
            else:
                iota_g = iota_f

            identity = const_pool.tile([P, P], f32)
            make_identity(nc, identity[:])

            wl_sb = const_pool.tile([D, D], f32)
            nc.sync.dma_start(wl_sb[:], wl[:])
            wr_sb = const_pool.tile([D, D], f32)
            nc.sync.dma_start(wr_sb[:], wr[:])
            bl_sb = const_pool.tile([1, D], f32)
            nc.sync.dma_start(bl_sb[:], bl[:])
            ones1 = const_pool.tile([1, D], f32)
            nc.vector.memset(ones1[:], 1.0)

            idx_all = meta_pool.tile([P, TILES_PER_CORE, NBANKS, IW], i16)
            nc.sync.dma_start(idx_all[:], gidx[:])
            dst_all = meta_pool.tile([P, TILES_PER_CORE, KT], f32)
            nc.sync.dma_start(dst_all[:], dstrel[:])
            w_all = meta_pool.tile([P, TILES_PER_CORE, KT], f32)
            nc.sync.dma_start(w_all[:], wgt[:])

            # ---- main loop over node tiles ----
            rep_ctx = (
                tc.For_i(0, bench_repeat, 1)
                if bench_repeat > 1
                else contextlib.nullcontext()
            )
            with rep_ctx:
              for t in range(TILES_PER_CORE):
                # gather x[src] per bank: msg position (p, j) <- edge j*128+p
                msgs = []
                if "gather" not in ablate_set:
                    nb = 2 if "2banks" in ablate_set else NBANKS
                    elem = D // 2 if "half" in ablate_set else D
                    estep = D if "half" in ablate_set else None
                    for b in range(nb):
                        xpad_v = xgat[b * BANK : (b + 1) * BANK, :elem]
                        msg = msg_pool.tile([P, KB, elem], gdt, tag="msg")
                        if "splitgather" in ablate_set:
                            k1 = 2  # chunks in first gather
                            nc.gpsimd.dma_gather(
                                out_ap=msg[:, :k1, :],
                                in_ap=xpad_v,
                                idxs_ap=idx_all[:, t, b, : k1 * 8],
                                num_idxs=k1 * P,
                                num_idxs_reg=k1 * P,
                                elem_size=elem,
                                elem_step=estep,
                                queue_num=b % NQUEUES,
                            )
                            nc.gpsimd.dma_gather(
                                out_ap=msg[:, k1:, :],
                                in_ap=xpad_v,
                                idxs_ap=idx_all[:, t, b, k1 * 8 :],
                                num_idxs=(KB - k1) * P,
                                num_idxs_reg=(KB - k1) * P,
                                elem_size=elem,
                                elem_step=estep,
                                queue_num=b % NQUEUES,
                            )
                        else:
                            nc.gpsimd.dma_gather(
                                out_ap=msg[:],
                                in_ap=xpad_v,
                                idxs_ap=idx_all[:, t, b, :],
                                num_idxs=NIDX,
                                num_idxs_reg=NIDX,
                                elem_size=elem,
                                elem_step=estep,
                                single_packet=SINGLE_PACKET,
                                queue_num=b % NQUEUES,
                            )
                        msgs.append(msg)

                # root path: x tile, transposed via PE
                x_sb = xt_pool.tile([P, D], f32, tag="x_in")
                nc.sync.dma_start(x_sb[:], xchunk[t * P : (t + 1) * P, :])
                ps_xt = ps_xt_pool.tile([P, P], f32)
                nc.tensor.transpose(out=ps_xt[:], in_=x_sb[:], identity=identity[:])
                xT_sb = xt_pool.tile([P, D], f32, tag="x_t")
                nc.scalar.copy(xT_sb[:], ps_xt[:])

                # aggregation: sumT/meanT[f, n] accumulated over chunk slots
                ps_mean = ps_mean_pool.tile([P, P], f32)
                for s in range(KT):
                    b, j = divmod(s, KB)
                    if "onehot" not in ablate_set:
                        oh = oh_pool.tile([P, P], gdt)
                        if GATHER_BF16:
                            nc.vector.tensor_scalar(
                                oh[:],
                                iota_g[:],
                                dst_all[:, t, s : s + 1],
                                None,
                                mybir.AluOpType.is_equal,
                            )
                        else:
                            nc.vector.tensor_scalar(
                                oh[:],
                                iota_g[:],
                                dst_all[:, t, s : s + 1],
                                w_all[:, t, s : s + 1],
                                mybir.AluOpType.is_equal,
                                mybir.AluOpType.mult,
                            )
                        rhs_ap = oh[:]
                    else:
                        rhs_ap = iota_g[:]
                    if "chunkmm" not in ablate_set:
                        lhs_ap = (
                            msgs[b][:, j, :]
                            if "gather" not in ablate_set
                            else iota_g[:]
                        )
                        nc.tensor.matmul(
                            out=ps_mean[:],
                            lhsT=lhs_ap,
                            rhs=rhs_ap,
                            start=(s == 0),
                            stop=(s == KT - 1),
                        )
                if "chunkmm" in ablate_set:
                    nc.tensor.matmul(
                        out=ps_mean[:],
                        lhsT=iota_g[:],
                        rhs=iota_g[:],
                        start=True,
                        stop=True,
                    )
                meanT_sb = ep_pool.tile([P, P], f32, tag="meanT")
                nc.scalar.copy(meanT_sb[:], ps_mean[:])

                if GATHER_BF16:
                    # ps_a = sumT.T @ W_l; scale rows by 1/deg (exact f32)
                    ps_a = ps_out_pool.tile([P, P], f32, tag="ps_a")
                    nc.tensor.matmul(
                        out=ps_a[:], lhsT=meanT_sb[:], rhs=wl_sb[:],
                        start=True, stop=True,
                    )
                    out_l = ep_pool.tile([P, P], f32, tag="out_l")
                    nc.vector.tensor_scalar(
                        out_l[:],
                        ps_a[:],
                        rcol_sb[:, t : t + 1],
                        None,
                        mybir.AluOpType.mult,
                    )
                    # ps_b = xT.T @ W_r + ones x b_l; final = out_l + ps_b
                    ps_o = ps_out_pool.tile([P, P], f32, tag="ps_b")
                    nc.tensor.matmul(
                        out=ps_o[:], lhsT=xT_sb[:], rhs=wr_sb[:],
                        start=True, stop=False,
                    )
                    nc.tensor.matmul(
                        out=ps_o[:], lhsT=ones1[:], rhs=bl_sb[:],
                        start=False, stop=True,
                    )
                    final = ep_pool.tile([P, P], f32, tag="final")
                    nc.vector.tensor_tensor(
                        out=final[:], in0=out_l[:], in1=ps_o[:],
                        op=mybir.AluOpType.add,
                    )
                    norm_src = final[:]
                else:
                    # linear: out[n,fo] = meanT.T @ W_l + xT.T @ W_r + ones x b_l
                    ps_o = ps_out_pool.tile([P, P], f32, tag="ps_b")
                    nc.tensor.matmul(
                        out=ps_o[:], lhsT=meanT_sb[:], rhs=wl_sb[:],
                        start=True, stop=False,
                    )
                    nc.tensor.matmul(
                        out=ps_o[:], lhsT=xT_sb[:], rhs=wr_sb[:],
                        start=False, stop=False,
                    )
                    nc.tensor.matmul(
                        out=ps_o[:], lhsT=ones1[:], rhs=bl_sb[:],
                        start=False, stop=True,
                    )
                    norm_src = ps_o[:]

                # row-wise L2 normalize: out / max(||out||, 1e-12)
                sq_scr = ep_pool.tile([P, P], f32, tag="sq")
                ss = ep_pool.tile([P, 1], f32, tag="ss")
                nc.scalar.activation(
                    sq_scr[:],
                    norm_src,
                    mybir.ActivationFunctionType.Square,
                    accum_out=ss[:],
                )
                nrm = ep_pool.tile([P, 1], f32, tag="nrm")
                nc.scalar.sqrt(nrm[:], ss[:])
                nrmc = ep_pool.tile([P, 1], f32, tag="nrmc")
                nc.vector.tensor_scalar_max(nrmc[:], nrm[:], 1e-12)
                rn = ep_pool.tile([P, 1], f32, tag="rn")
                nc.vector.reciprocal(rn[:], nrmc[:])

                out_sb = ep_pool.tile([P, P], f32, tag="out")
                nc.vector.tensor_scalar(
                    out_sb[:],
                    norm_src,
                    rn[:, :1],
                    None,
                    mybir.AluOpType.mult,
                )
                nc.sync.dma_start(out[t * P : (t + 1) * P, :], out_sb[:])

    nc.compile()
    return nc


def _prepare(x, edge_index):
    """Host-side sharding: sort by dst, group per (tile, bank), pack chunks."""
    src = np.ascontiguousarray(edge_index[0]).astype(np.int64)
    dst = np.ascontiguousarray(edge_index[1]).astype(np.int64)

    cnt = np.bincount(dst, minlength=N_NODES)
    w_node = (1.0 / np.maximum(cnt, 1)).astype(np.float32)

    order = np.argsort(dst, kind="stable")
    src_s = src[order]
    dst_s = dst[order]

    # per-core edge ranges and per (core,tile,bank) grouping
    per_core = []
    KB = 1
    for c in range(N_CORES):
        base = c * NODES_PER_CORE
        lo = np.searchsorted(dst_s, base)
        hi = np.searchsorted(dst_s, base + NODES_PER_CORE)
        s_c = src_s[lo:hi]
        d_c = dst_s[lo:hi] - base
        t_c = d_c // P
        b_c = s_c // BANK
        key = (t_c * NBANKS + b_c).astype(np.int64)
        ordc = np.argsort(key, kind="stable")
        s_c, d_c, key = s_c[ordc], d_c[ordc], key[ordc]
        counts = np.bincount(key, minlength=TILES_PER_CORE * NBANKS)
        KB = max(KB, int(np.ceil(counts.max() / P)))
        per_core.append((s_c, d_c, counts))

    KT = NBANKS * KB
    NIDX = KB * P
    IW = NIDX // 16

    # per-node 1/max(deg,1) as [core][lane, tile] columns
    wg = np.ones(X_PAD_ROWS, np.float32)
    wg[:N_NODES] = w_node
    rcol = np.zeros((N_CORES, P, TILES_PER_CORE), np.float32)
    for c in range(N_CORES):
        idx = (
            c * NODES_PER_CORE
            + (np.arange(TILES_PER_CORE) * P)[None, :]
            + np.arange(P)[:, None]
        )
        rcol[c] = wg[idx]

    gidx = np.zeros((N_CORES, P, TILES_PER_CORE, NBANKS, IW), np.int16)
    dstrel = np.full((N_CORES, P, TILES_PER_CORE, KT), -1.0, np.float32)
    wgt = np.zeros((N_CORES, P, TILES_PER_CORE, KT), np.float32)

    prow = np.arange(P) % 16
    scol = np.arange(IW) * 16
    for c in range(N_CORES):
        s_c, d_c, counts = per_core[c]
        starts = np.concatenate([[0], np.cumsum(counts)])
        for t in range(TILES_PER_CORE):
            for b in range(NBANKS):
                g = t * NBANKS + b
                n = counts[g]
                if n == 0:
                    continue
                lo = starts[g]
                sv = s_c[lo : lo + n] - b * BANK
                dv = (d_c[lo : lo + n] - t * P).astype(np.float32)
                wv = w_node[d_c[lo : lo + n] + c * NODES_PER_CORE]
                i_pad = np.zeros(NIDX, np.int16)
                i_pad[:n] = sv.astype(np.int16)
                d_pad = np.full(NIDX, -1.0, np.float32)
                d_pad[:n] = dv
                w_pad = np.zeros(NIDX, np.float32)
                w_pad[:n] = wv
                # idx position i lives at [i % 16, i // 16], replicated %16
                gidx[c, :, t, b, :] = i_pad[scol[None, :] + prow[:, None]]
                # chunk slot s=b*KB+j, lane p <- edge j*128+p
                dstrel[c, :, t, b * KB : (b + 1) * KB] = d_pad.reshape(KB, P).T
                wgt[c, :, t, b * KB : (b + 1) * KB] = w_pad.reshape(KB, P).T

    return gidx, dstrel, wgt, rcol, KB


def kernel(x, edge_index, W_l, b_l, W_r):
    from concourse.bass_utils import run_bass_kernel_spmd

    x = np.ascontiguousarray(np.asarray(x, dtype=np.float32))
    W_l = np.ascontiguousarray(np.asarray(W_l, dtype=np.float32))
    W_r = np.ascontiguousarray(np.asarray(W_r, dtype=np.float32))
    b_l = np.ascontiguousarray(np.asarray(b_l, dtype=np.float32)).reshape(1, D)

    gidx, dstrel, wgt, rcol, KB = _prepare(x, np.asarray(edge_index))

    xpad = np.zeros((X_PAD_ROWS, D), np.float32)
    xpad[:N_NODES] = x
    if GATHER_BF16:
        import ml_dtypes

        xbf = xpad.astype(ml_dtypes.bfloat16)

    if KB not in _program_cache:
        _program_cache[KB] = _build_program(KB)
    nc = _program_cache[KB]

    in_maps = []
    for c in range(N_CORES):
        base = c * NODES_PER_CORE
        m = {
            "xchunk": xpad[base : base + NODE_PAD],
            "gidx": gidx[c],
            "dstrel": dstrel[c],
            "wgt": wgt[c],
            "wl": W_l,
            "wr": W_r,
            "bl": b_l,
        }
        if GATHER_BF16:
            m["xbf"] = xbf
            m["rcol"] = rcol[c]
        else:
            m["xpad"] = xpad
        in_maps.append(m)

    LAST["nc"] = nc
    LAST["in_maps"] = in_maps
    r = run_bass_kernel_spmd(nc, in_maps, list(range(N_CORES)), trace=TRACE)
    LAST["exec_time_ns"] = r.exec_time_ns
    res = r.results
    out = np.concatenate(
        [res[c]["out"][:NODES_PER_CORE] for c in range(N_CORES)], axis=0
    )
    return out



# revision 18
# speedup vs baseline: 71.8575x; 71.8575x over previous
"""BipartiteSAGEConv on 8 Trainium2 NeuronCores.

out = normalize(mean_{dst}(x[src]) @ W_l + b_l + x @ W_r)

Strategy (v3):
- Host: shard destination nodes across the 8 cores (each core owns 12500
  contiguous nodes and all edges pointing into them -> no cross-core
  reduction). Within a core, nodes are PACKED into 98 tiles of 128 by a
  greedy bin-packer so that each (tile, src-bank) edge group fits a fixed
  rotating chunk template KB[t][b] = 5 if b == t%4 else 4 (17 chunks/tile,
  213k gather indices vs 251k for the uniform 5,5,5,5 template). The
  per-edge weight w = 1/max(deg(dst),1) is folded into the bf16 one-hot so
  PSUM accumulation yields the mean directly.
- Device (SPMD, identical program on all 8 cores):
  * SWDGE dma_gather of x[src] rows (bf16, 256B each) from 4 HBM banks of
    25024 rows (int16 index limit). Descriptor generation on the GPSIMD Q7
    cores is the kernel's critical path (~4ns/idx, fully serial), so total
    index count is what matters. Calls are capped at 7 chunks = 896 idx
    (57 descs/DMA, under the fixed 64-desc SWDGE ring; more hangs the HW)
    and rotate across 4 queues so transfers drain between same-queue calls.
    Indices stream from HBM per 7-tile segment (double-buffered).
  * DVE builds the weighted one-hot (iota == dstrel) * w; PE accumulates
    meanT[f, n] += msg[e, f].T @ onehot[e, n] over the tile's 17 chunks,
    then one PSUM group: out = meanT.T @ W_l + xT.T @ W_r + ones x b_l,
    with xT pre-transposed on the host.
  * Results collect in one SBUF buffer [128, 98, 128]; row sum-of-squares
    per tile via ACT Square+accum; sqrt/clamp/reciprocal once on [128, 98];
    final scale per tile; output rows are un-permuted on the host.
"""

import numpy as np

N_NODES = 100000
D = 128
P = 128
N_CORES = 8
NODES_PER_CORE = N_NODES // N_CORES  # 12500
TILES_PER_CORE = (NODES_PER_CORE + P - 1) // P  # 98
NODE_PAD = TILES_PER_CORE * P  # 12544
X_PAD_ROWS = 100096  # 782 * 128
BANK = X_PAD_ROWS // 4  # 25024 rows per gather bank (< 32768 int16 limit)
NBANKS = 4
G = 7  # tiles per segment (98 = 14 * 7)
NBATCH = TILES_PER_CORE // G  # 14
MAXCH = 7  # max chunks per dma_gather call (896 idx = 57 descs <= 64 ring)

NQUEUES = 4
SCRATCH = 16384
SINGLE_PACKET = False

_program_cache = {}

# test harness hooks
TRACE = False
LAST = {}


def _kbmat(template):
    """Per-(tile, bank) chunk counts for a template id."""
    kb = np.zeros((TILES_PER_CORE, NBANKS), np.int64)
    for t in range(TILES_PER_CORE):
        for b in range(NBANKS):
            if template == "5444":
                kb[t, b] = 5 if b == t % 4 else 4
            elif template == "5544":
                kb[t, b] = 5 if b in (t % 4, (t + 1) % 4) else 4
            else:  # "5555"
                kb[t, b] = 5
    return kb


def _derive(kb):
    """Static layout bookkeeping from the chunk-count matrix."""
    KT = int(kb[0].sum())  # same for every tile by construction
    assert (kb.sum(1) == KT).all()
    slot_off = np.concatenate(
        [np.zeros((TILES_PER_CORE, 1), np.int64), np.cumsum(kb, 1)], 1
    )  # [T, 5]
    chunk_off = np.concatenate(
        [np.zeros((1, NBANKS), np.int64), np.cumsum(kb, 0)], 0
    )  # [T+1, B] per-bank chunk prefix over tiles
    CB = chunk_off[-1]  # chunks per bank
    OFFB = np.concatenate([[0], np.cumsum(CB)])  # bank col-block offsets (chunks)
    # per segment: chunk ranges and call partition
    segs = []
    MW = 0
    for g in range(NBATCH):
        per_bank = []
        for b in range(NBANKS):
            c0 = int(chunk_off[g * G, b])
            c1 = int(chunk_off[(g + 1) * G, b])
            cg = c1 - c0
            MW = max(MW, cg)
            npieces = -(-cg // MAXCH)
            base, extra = divmod(cg, npieces)
            sizes = [base + (1 if i < extra else 0) for i in range(npieces)]
            per_bank.append((c0, cg, sizes))
        segs.append(per_bank)
    return KT, slot_off, chunk_off, CB, OFFB, segs, MW


def _build_program(template, ablate="", bench_repeat=1):
    """Build + compile the SPMD Bass program for a chunk template."""
    ablate_set = set(ablate.split(",")) if ablate else set()
    import contextlib

    import concourse.bass as bass
    import concourse.tile as tile
    from concourse import bacc, mybir

    f32 = mybir.dt.float32
    bf16 = mybir.dt.bfloat16
    i16 = mybir.dt.int16

    kb = _kbmat(template)
    KT, slot_off, chunk_off, CB, OFFB, segs, MW = _derive(kb)
    TOTCH = int(CB.sum())

    nc = bacc.Bacc(
        "TRN2",
        target_bir_lowering=False,
        debug=False,
        num_devices=N_CORES,
        num_swdge_queues=NQUEUES,
        dynamic_dma_scratch_size=SCRATCH,
    )

    xbf = nc.dram_tensor("xbf", [X_PAD_ROWS, D], bf16, kind="ExternalInput")
    xT_in = nc.dram_tensor("xT", [D, NODE_PAD], f32, kind="ExternalInput")
    gidx = nc.dram_tensor("gidx", [P, TOTCH * 8], i16, kind="ExternalInput")
    dstrel = nc.dram_tensor("dstrel", [P, TILES_PER_CORE, KT], f32, kind="ExternalInput")
    wgt = nc.dram_tensor("wgt", [P, TILES_PER_CORE, KT], f32, kind="ExternalInput")
    iota_in = nc.dram_tensor("iota", [P, P], bf16, kind="ExternalInput")
    wl = nc.dram_tensor("wl", [D, D], f32, kind="ExternalInput")
    wr = nc.dram_tensor("wr", [D, D], f32, kind="ExternalInput")
    bl = nc.dram_tensor("bl", [1, D], f32, kind="ExternalInput")
    out = nc.dram_tensor("out", [NODE_PAD, D], f32, kind="ExternalOutput")

    with tile.TileContext(nc) as tc:
        with (
            tc.tile_pool(name="const", bufs=1) as const_pool,
            tc.tile_pool(name="meta", bufs=1) as meta_pool,
            tc.tile_pool(name="idx", bufs=2) as idx_pool,
            tc.tile_pool(name="msg", bufs=2) as msg_pool,
            tc.tile_pool(name="oh", bufs=6) as oh_pool,
            tc.tile_pool(name="ep", bufs=3) as ep_pool,
            tc.tile_pool(name="ps_mean", bufs=2, space="PSUM") as ps_mean_pool,
            tc.tile_pool(name="ps_out", bufs=2, space="PSUM") as ps_out_pool,
        ):
            # ---- constants / weights / metadata (loaded once) ----
            iota_g = const_pool.tile([P, P], bf16)
            nc.sync.dma_start(iota_g[:], iota_in[:])
            wl_sb = const_pool.tile([D, D], f32)
            nc.sync.dma_start(wl_sb[:], wl[:])
            wr_sb = const_pool.tile([D, D], f32)
            nc.sync.dma_start(wr_sb[:], wr[:])
            bl_sb = const_pool.tile([1, D], f32)
            nc.sync.dma_start(bl_sb[:], bl[:])
            ones1 = const_pool.tile([1, D], f32)
            nc.vector.memset(ones1[:], 1.0)

            xT_all = const_pool.tile([D, NODE_PAD], f32)
            nc.sync.dma_start(xT_all[:], xT_in[:])
            dst_all = meta_pool.tile([P, TILES_PER_CORE, KT], f32)
            nc.sync.dma_start(dst_all[:], dstrel[:])
            w_all = meta_pool.tile([P, TILES_PER_CORE, KT], f32)
            nc.sync.dma_start(w_all[:], wgt[:])

            obuf = const_pool.tile([P, TILES_PER_CORE, P], f32)
            nbuf = const_pool.tile([P, TILES_PER_CORE], f32)

            idxbs = [
                idx_pool.tile([P, NBANKS, MW * 8], i16, tag="idx", name=f"idxb{i}")
                for i in range(2)
            ]

            def load_idx(g, buf):
                for b in range(NBANKS):
                    c0, cg, _ = segs[g][b]
                    nc.sync.dma_start(
                        buf[:, b, : cg * 8],
                        gidx[:, (OFFB[b] + c0) * 8 : (OFFB[b] + c0 + cg) * 8],
                    )

            load_idx(0, idxbs[0])

            if "gather" in ablate_set:
                msg_fixed = [
                    msg_pool.tile(
                        [P, NBANKS, MW, D], bf16, tag="msg", name=f"msgf{i}"
                    )
                    for i in range(2)
                ]
                for mt in msg_fixed:
                    nc.vector.memset(mt[:], 0.01)

            rep_ctx = (
                tc.For_i(0, bench_repeat, 1)
                if bench_repeat > 1
                else contextlib.nullcontext()
            )
            rep_ctx.__enter__()
            for g in range(NBATCH):
                if g + 1 < NBATCH:
                    load_idx(g + 1, idxbs[(g + 1) % 2])
                if "gather" in ablate_set:
                    msg = msg_fixed[g % 2]
                else:
                    msg = msg_pool.tile([P, NBANKS, MW, D], bf16, tag="msg")
                    # emit calls round-robin across banks (= queues) so
                    # same-queue calls are ~4 desc-gen slots apart and each
                    # transfer drains before its queue's ring is reused
                    maxp = max(len(segs[g][b][2]) for b in range(NBANKS))
                    for pi in range(maxp):
                        for b in range(NBANKS):
                            c0, cg, sizes = segs[g][b]
                            if pi >= len(sizes):
                                continue
                            p0 = sum(sizes[:pi])
                            pn = sizes[pi]
                            nc.gpsimd.dma_gather(
                                out_ap=msg[:, b, p0 : p0 + pn, :],
                                in_ap=xbf[b * BANK : (b + 1) * BANK, :],
                                idxs_ap=idxbs[g % 2][:, b, p0 * 8 : (p0 + pn) * 8],
                                num_idxs=pn * P,
                                num_idxs_reg=pn * P,
                                elem_size=D,
                                single_packet=SINGLE_PACKET,
                                queue_num=b % NQUEUES,
                            )
                for tl in range(G):
                    t = g * G + tl
                    ps_mean = ps_mean_pool.tile([P, P], f32)
                    for s in range(KT):
                        # slot s -> (bank, chunk-in-seg)
                        b = int(np.searchsorted(slot_off[t], s, side="right")) - 1
                        j = s - int(slot_off[t][b])
                        cpos = int(chunk_off[t][b] - chunk_off[g * G][b]) + j
                        oh = oh_pool.tile([P, P], bf16)
                        nc.vector.tensor_scalar(
                            oh[:],
                            iota_g[:],
                            dst_all[:, t, s : s + 1],
                            w_all[:, t, s : s + 1],
                            mybir.AluOpType.is_equal,
                            mybir.AluOpType.mult,
                        )
                        nc.tensor.matmul(
                            out=ps_mean[:],
                            lhsT=msg[:, b, cpos, :],
                            rhs=oh[:],
                            start=(s == 0),
                            stop=(s == KT - 1),
                        )
                    meanT_sb = ep_pool.tile([P, P], f32, tag="meanT")
                    nc.scalar.copy(meanT_sb[:], ps_mean[:])

                    ps_o = ps_out_pool.tile([P, P], f32)
                    nc.tensor.matmul(
                        out=ps_o[:], lhsT=meanT_sb[:], rhs=wl_sb[:],
                        start=True, stop=False,
                    )
                    nc.tensor.matmul(
                        out=ps_o[:],
                        lhsT=xT_all[:, t * P : (t + 1) * P],
                        rhs=wr_sb[:],
                        start=False, stop=False,
                    )
                    nc.tensor.matmul(
                        out=ps_o[:], lhsT=ones1[:], rhs=bl_sb[:],
                        start=False, stop=True,
                    )
                    nc.vector.tensor_copy(obuf[:, t, :], ps_o[:])
                    sq_scr = ep_pool.tile([P, P], f32, tag="sq")
                    nc.scalar.activation(
                        sq_scr[:],
                        ps_o[:],
                        mybir.ActivationFunctionType.Square,
                        accum_out=nbuf[:, t : t + 1],
                    )

            # ---- epilogue: row-wise L2 normalize, store ----
            nrm = ep_pool.tile([P, TILES_PER_CORE], f32, tag="nrm")
            nc.scalar.sqrt(nrm[:], nbuf[:])
            nrmc = ep_pool.tile([P, TILES_PER_CORE], f32, tag="nrmc")
            nc.vector.tensor_scalar_max(nrmc[:], nrm[:], 1e-12)
            rn = ep_pool.tile([P, TILES_PER_CORE], f32, tag="rn")
            nc.vector.reciprocal(rn[:], nrmc[:])
            for t in range(TILES_PER_CORE):
                nc.vector.tensor_scalar(
                    obuf[:, t, :],
                    obuf[:, t, :],
                    rn[:, t : t + 1],
                    None,
                    mybir.AluOpType.mult,
                )
                nc.sync.dma_start(out[t * P : (t + 1) * P, :], obuf[:, t, :])
            rep_ctx.__exit__(None, None, None)

    nc.compile()
    return nc


def _pack_core(vb, kb):
    """Greedy node->tile packing under per-(tile,bank) edge capacities.

    vb: [n_nodes, NBANKS] per-node per-bank in-degree. Returns assign
    [n_nodes] tile ids, or None if infeasible.
    """
    n = vb.shape[0]
    cap = kb * P  # [T, B] edge capacity
    rem = cap.astype(np.int64).copy()
    slots = np.full(TILES_PER_CORE, P, np.int64)
    assign = np.full(n, -1, np.int64)
    order = np.argsort(-vb.sum(1), kind="stable")
    for node in order:
        v = vb[node]
        ok = (slots > 0) & (rem >= v[None, :]).all(1)
        if not ok.any():
            return None
        score = (rem - v[None, :]).min(1)
        score[~ok] = -(1 << 40)
        t = int(np.argmax(score))
        assign[node] = t
        rem[t] -= v
        slots[t] -= 1
    return assign


def _prepare(edge_index):
    """Host-side sharding: shard by dst range, pack tiles, build gather meta."""
    src = np.ascontiguousarray(edge_index[0]).astype(np.int64)
    dst = np.ascontiguousarray(edge_index[1]).astype(np.int64)

    cnt = np.bincount(dst, minlength=N_NODES)
    w_node = (1.0 / np.maximum(cnt, 1)).astype(np.float32)

    order = np.argsort(dst, kind="stable")
    src_s = src[order]
    dst_s = dst[order]

    core_edges = []
    for c in range(N_CORES):
        base = c * NODES_PER_CORE
        lo = np.searchsorted(dst_s, base)
        hi = np.searchsorted(dst_s, base + NODES_PER_CORE)
        core_edges.append((src_s[lo:hi], dst_s[lo:hi] - base))

    # try templates cheapest-first; all cores must pack under the same one
    for template in ("5444", "5544", "5555"):
        kb = _kbmat(template)
        assigns = []
        for c in range(N_CORES):
            s_c, d_c = core_edges[c]
            vb = np.zeros((NODES_PER_CORE, NBANKS), np.int64)
            np.add.at(vb, (d_c, s_c // BANK), 1)
            a = _pack_core(vb, kb)
            if a is None:
                break
            assigns.append(a)
        if len(assigns) == N_CORES:
            break
    KT, slot_off, chunk_off, CB, OFFB, segs, MW = _derive(kb)
    TOTCH = int(CB.sum())

    gidx = np.zeros((N_CORES, P, TOTCH * 8), np.int16)
    dstrel = np.full((N_CORES, P, TILES_PER_CORE, KT), -1.0, np.float32)
    wgt = np.zeros((N_CORES, P, TILES_PER_CORE, KT), np.float32)
    perm = np.full((N_CORES, NODE_PAD), -1, np.int64)  # slot -> local node id

    prow = np.arange(P) % 16
    for c in range(N_CORES):
        s_c, d_c = core_edges[c]
        a = assigns[c]
        # position of each node within its tile (by assignment order)
        tile_order = np.argsort(a, kind="stable")
        pos = np.zeros(NODES_PER_CORE, np.int64)
        tcounts = np.bincount(a, minlength=TILES_PER_CORE)
        starts = np.concatenate([[0], np.cumsum(tcounts)])
        for t in range(TILES_PER_CORE):
            nodes_t = tile_order[starts[t] : starts[t + 1]]
            pos[nodes_t] = np.arange(len(nodes_t))
            perm[c, t * P : t * P + len(nodes_t)] = nodes_t
        # edges -> (tile, bank) groups
        et = a[d_c]
        eb = s_c // BANK
        key = et * NBANKS + eb
        ordc = np.argsort(key, kind="stable")
        s_e, d_e = s_c[ordc], d_c[ordc]
        counts = np.bincount(key[ordc], minlength=TILES_PER_CORE * NBANKS)
        gstarts = np.concatenate([[0], np.cumsum(counts)])
        for t in range(TILES_PER_CORE):
            for b in range(NBANKS):
                gkey = t * NBANKS + b
                n = counts[gkey]
                nslots = kb[t, b] * P
                assert n <= nslots, (t, b, n, nslots)
                lo = gstarts[gkey]
                sv = s_e[lo : lo + n] - b * BANK
                dv = pos[d_e[lo : lo + n]].astype(np.float32)
                wv = w_node[d_e[lo : lo + n] + c * NODES_PER_CORE]
                i_pad = np.zeros(nslots, np.int16)
                i_pad[:n] = sv.astype(np.int16)
                d_pad = np.full(nslots, -1.0, np.float32)
                d_pad[:n] = dv
                w_pad = np.zeros(nslots, np.float32)
                w_pad[:n] = wv
                # bank-stream columns for this group's chunks
                col0 = (OFFB[b] + chunk_off[t, b]) * 8
                ncol = kb[t, b] * 8
                scol = np.arange(ncol) * 16
                gidx[c, :, col0 : col0 + ncol] = i_pad[
                    scol[None, :] + prow[:, None]
                ]
                s0 = slot_off[t][b]
                dstrel[c, :, t, s0 : s0 + kb[t, b]] = d_pad.reshape(-1, P).T
                wgt[c, :, t, s0 : s0 + kb[t, b]] = w_pad.reshape(-1, P).T

    return gidx, dstrel, wgt, perm, template


def kernel(x, edge_index, W_l, b_l, W_r):
    import ml_dtypes
    from concourse.bass_utils import run_bass_kernel_spmd

    x = np.ascontiguousarray(np.asarray(x, dtype=np.float32))
    W_l = np.ascontiguousarray(np.asarray(W_l, dtype=np.float32))
    W_r = np.ascontiguousarray(np.asarray(W_r, dtype=np.float32))
    b_l = np.ascontiguousarray(np.asarray(b_l, dtype=np.float32)).reshape(1, D)

    gidx, dstrel, wgt, perm, template = _prepare(np.asarray(edge_index))

    xpad = np.zeros((X_PAD_ROWS, D), np.float32)
    xpad[:N_NODES] = x
    xbf = xpad.astype(ml_dtypes.bfloat16)
    iota = np.broadcast_to(
        np.arange(P, dtype=np.float32)[None, :], (P, P)
    ).astype(ml_dtypes.bfloat16)

    if template not in _program_cache:
        _program_cache[template] = _build_program(template)
    nc = _program_cache[template]

    in_maps = []
    for c in range(N_CORES):
        base = c * NODES_PER_CORE
        pc = perm[c]
        xperm = np.zeros((NODE_PAD, D), np.float32)
        valid = pc >= 0
        xperm[valid] = x[base + pc[valid]]
        m = {
            "xbf": xbf,
            "xT": np.ascontiguousarray(xperm.T),
            "gidx": gidx[c],
            "dstrel": dstrel[c],
            "wgt": wgt[c],
            "iota": np.ascontiguousarray(iota),
            "wl": W_l,
            "wr": W_r,
            "bl": b_l,
        }
        in_maps.append(m)

    LAST["nc"] = nc
    LAST["in_maps"] = in_maps
    LAST["perm"] = perm
    r = run_bass_kernel_spmd(nc, in_maps, list(range(N_CORES)), trace=TRACE)
    LAST["exec_time_ns"] = r.exec_time_ns
    res = r.results
    return unpermute(perm, [res[c]["out"] for c in range(N_CORES)])


def unpermute(perm, raw):
    """Map device output rows (tile-slot order) back to node order."""
    out = np.empty((N_NODES, D), np.float32)
    for c in range(N_CORES):
        base = c * NODES_PER_CORE
        pc = perm[c]
        valid = pc >= 0
        out[base + pc[valid]] = raw[c][valid]
    return out


# revision 22
# speedup vs baseline: 75.4565x; 1.0501x over previous
"""BipartiteSAGEConv on 8 Trainium2 NeuronCores.

out = normalize(mean_{dst}(x[src]) @ W_l + b_l + x @ W_r)

Strategy (v3):
- Host: shard destination nodes across the 8 cores (each core owns 12500
  contiguous nodes and all edges pointing into them -> no cross-core
  reduction). Within a core, nodes are PACKED into 98 tiles of 128 by a
  greedy bin-packer so that each (tile, src-bank) edge group fits a fixed
  rotating chunk template KB[t][b] = 5 if b == t%4 else 4 (17 chunks/tile,
  213k gather indices vs 251k for the uniform 5,5,5,5 template). The
  per-edge weight w = 1/max(deg(dst),1) is folded into the bf16 one-hot so
  PSUM accumulation yields the mean directly.
- Device (SPMD, identical program on all 8 cores):
  * SWDGE dma_gather of x[src] rows (bf16, 256B each) from 4 HBM banks of
    25024 rows (int16 index limit). Descriptor generation on the GPSIMD Q7
    cores is the kernel's critical path (~4ns/idx, fully serial), so total
    index count is what matters. Calls are capped at 7 chunks = 896 idx
    (57 descs/DMA, under the fixed 64-desc SWDGE ring; more hangs the HW)
    and rotate across 4 queues so transfers drain between same-queue calls.
    Indices stream from HBM per 7-tile segment (double-buffered).
  * DVE builds the weighted one-hot (iota == dstrel) * w; PE accumulates
    meanT[f, n] += msg[e, f].T @ onehot[e, n] over the tile's 17 chunks,
    then one PSUM group: out = meanT.T @ W_l + xT.T @ W_r + ones x b_l,
    with xT pre-transposed on the host.
  * Results collect in one SBUF buffer [128, 98, 128]; row sum-of-squares
    per tile via ACT Square+accum; sqrt/clamp/reciprocal once on [128, 98];
    final scale per tile; output rows are un-permuted on the host.
"""

import numpy as np

N_NODES = 100000
D = 128
P = 128
N_CORES = 8
NODES_PER_CORE = N_NODES // N_CORES  # 12500
TILES_PER_CORE = (NODES_PER_CORE + P - 1) // P  # 98
NODE_PAD = TILES_PER_CORE * P  # 12544
X_PAD_ROWS = 100096  # 782 * 128
BANK = X_PAD_ROWS // 4  # 25024 rows per gather bank (< 32768 int16 limit)
NBANKS = 4
G = 7  # tiles per segment (98 = 14 * 7)
NBATCH = TILES_PER_CORE // G  # 14
MAXCH = 7  # max chunks per dma_gather call (896 idx = 57 descs <= 64 ring)

NQUEUES = 4
SCRATCH = 16384
SINGLE_PACKET = False

_program_cache = {}

# test harness hooks
TRACE = False
LAST = {}


def _kbmat(template):
    """Per-(tile, bank) chunk counts for a template id."""
    kb = np.zeros((TILES_PER_CORE, NBANKS), np.int64)
    for t in range(TILES_PER_CORE):
        for b in range(NBANKS):
            if template == "seg5444":
                # fat bank fixed per segment -> (seg, bank) chunk counts are
                # 35 or 28, both of which split into fewer max-7-chunk calls
                kb[t, b] = 5 if b == (t // G) % 4 else 4
            elif template == "5444":
                kb[t, b] = 5 if b == t % 4 else 4
            elif template == "5544":
                kb[t, b] = 5 if b in (t % 4, (t + 1) % 4) else 4
            else:  # "5555"
                kb[t, b] = 5
    return kb


def _derive(kb):
    """Static layout bookkeeping from the chunk-count matrix."""
    KT = int(kb[0].sum())  # same for every tile by construction
    assert (kb.sum(1) == KT).all()
    slot_off = np.concatenate(
        [np.zeros((TILES_PER_CORE, 1), np.int64), np.cumsum(kb, 1)], 1
    )  # [T, 5]
    chunk_off = np.concatenate(
        [np.zeros((1, NBANKS), np.int64), np.cumsum(kb, 0)], 0
    )  # [T+1, B] per-bank chunk prefix over tiles
    CB = chunk_off[-1]  # chunks per bank
    OFFB = np.concatenate([[0], np.cumsum(CB)])  # bank col-block offsets (chunks)
    # per segment: chunk ranges and call partition
    segs = []
    MW = 0
    for g in range(NBATCH):
        per_bank = []
        for b in range(NBANKS):
            c0 = int(chunk_off[g * G, b])
            c1 = int(chunk_off[(g + 1) * G, b])
            cg = c1 - c0
            MW = max(MW, cg)
            npieces = -(-cg // MAXCH)
            base, extra = divmod(cg, npieces)
            sizes = [base + (1 if i < extra else 0) for i in range(npieces)]
            per_bank.append((c0, cg, sizes))
        segs.append(per_bank)
    return KT, slot_off, chunk_off, CB, OFFB, segs, MW


def _build_program(template, ablate="", bench_repeat=1):
    """Build + compile the SPMD Bass program for a chunk template."""
    ablate_set = set(ablate.split(",")) if ablate else set()
    import contextlib

    import concourse.bass as bass
    import concourse.tile as tile
    from concourse import bacc, mybir

    f32 = mybir.dt.float32
    bf16 = mybir.dt.bfloat16
    i16 = mybir.dt.int16

    kb = _kbmat(template)
    KT, slot_off, chunk_off, CB, OFFB, segs, MW = _derive(kb)
    TOTCH = int(CB.sum())

    nc = bacc.Bacc(
        "TRN2",
        target_bir_lowering=False,
        debug=False,
        num_devices=N_CORES,
        num_swdge_queues=NQUEUES,
        dynamic_dma_scratch_size=SCRATCH,
    )

    xbf = nc.dram_tensor("xbf", [X_PAD_ROWS, D], bf16, kind="ExternalInput")
    xT_in = nc.dram_tensor("xT", [D, NODE_PAD], f32, kind="ExternalInput")
    gidx = nc.dram_tensor("gidx", [P, TOTCH * 8], i16, kind="ExternalInput")
    dstrel = nc.dram_tensor("dstrel", [P, TILES_PER_CORE, KT], f32, kind="ExternalInput")
    wgt = nc.dram_tensor("wgt", [P, TILES_PER_CORE, KT], f32, kind="ExternalInput")
    iota_in = nc.dram_tensor("iota", [P, P], bf16, kind="ExternalInput")
    wl = nc.dram_tensor("wl", [D, D], f32, kind="ExternalInput")
    wr = nc.dram_tensor("wr", [D, D], f32, kind="ExternalInput")
    bl = nc.dram_tensor("bl", [1, D], f32, kind="ExternalInput")
    out = nc.dram_tensor("out", [NODE_PAD, D], f32, kind="ExternalOutput")

    with tile.TileContext(nc) as tc:
        with (
            tc.tile_pool(name="const", bufs=1) as const_pool,
            tc.tile_pool(name="meta", bufs=1) as meta_pool,
            tc.tile_pool(name="idx", bufs=2) as idx_pool,
            tc.tile_pool(name="msg", bufs=2) as msg_pool,
            tc.tile_pool(name="oh", bufs=6) as oh_pool,
            tc.tile_pool(name="ep", bufs=3) as ep_pool,
            tc.tile_pool(name="ps_mean", bufs=2, space="PSUM") as ps_mean_pool,
            tc.tile_pool(name="ps_out", bufs=2, space="PSUM") as ps_out_pool,
        ):
            # ---- constants / weights / metadata (loaded once) ----
            iota_g = const_pool.tile([P, P], bf16)
            nc.sync.dma_start(iota_g[:], iota_in[:])
            wl_sb = const_pool.tile([D, D], f32)
            nc.sync.dma_start(wl_sb[:], wl[:])
            wr_sb = const_pool.tile([D, D], f32)
            nc.sync.dma_start(wr_sb[:], wr[:])
            bl_sb = const_pool.tile([1, D], f32)
            nc.sync.dma_start(bl_sb[:], bl[:])
            ones1 = const_pool.tile([1, D], f32)
            nc.vector.memset(ones1[:], 1.0)

            xT_all = const_pool.tile([D, NODE_PAD], f32)
            nc.sync.dma_start(xT_all[:], xT_in[:])
            dst_all = meta_pool.tile([P, TILES_PER_CORE, KT], f32)
            nc.sync.dma_start(dst_all[:], dstrel[:])
            w_all = meta_pool.tile([P, TILES_PER_CORE, KT], f32)
            nc.sync.dma_start(w_all[:], wgt[:])

            obuf = const_pool.tile([P, TILES_PER_CORE, P], f32)
            nbuf = const_pool.tile([P, TILES_PER_CORE], f32)

            idxbs = [
                idx_pool.tile([P, NBANKS, MW * 8], i16, tag="idx", name=f"idxb{i}")
                for i in range(2)
            ]

            def load_idx(g, buf):
                for b in range(NBANKS):
                    c0, cg, _ = segs[g][b]
                    nc.sync.dma_start(
                        buf[:, b, : cg * 8],
                        gidx[:, (OFFB[b] + c0) * 8 : (OFFB[b] + c0 + cg) * 8],
                    )

            load_idx(0, idxbs[0])

            if "gather" in ablate_set:
                msg_fixed = [
                    msg_pool.tile(
                        [P, NBANKS, MW, D], bf16, tag="msg", name=f"msgf{i}"
                    )
                    for i in range(2)
                ]
                for mt in msg_fixed:
                    nc.vector.memset(mt[:], 0.01)

            rep_ctx = (
                tc.For_i(0, bench_repeat, 1)
                if bench_repeat > 1
                else contextlib.nullcontext()
            )
            rep_ctx.__enter__()
            for g in range(NBATCH):
                if g + 1 < NBATCH:
                    load_idx(g + 1, idxbs[(g + 1) % 2])
                if "gather" in ablate_set:
                    msg = msg_fixed[g % 2]
                else:
                    msg = msg_pool.tile([P, NBANKS, MW, D], bf16, tag="msg")
                    # emit calls round-robin across banks (= queues) so
                    # same-queue calls are ~4 desc-gen slots apart and each
                    # transfer drains before its queue's ring is reused
                    maxp = max(len(segs[g][b][2]) for b in range(NBANKS))
                    for pi in range(maxp):
                        for b in range(NBANKS):
                            c0, cg, sizes = segs[g][b]
                            if pi >= len(sizes):
                                continue
                            p0 = sum(sizes[:pi])
                            pn = sizes[pi]
                            nc.gpsimd.dma_gather(
                                out_ap=msg[:, b, p0 : p0 + pn, :],
                                in_ap=xbf[b * BANK : (b + 1) * BANK, :],
                                idxs_ap=idxbs[g % 2][:, b, p0 * 8 : (p0 + pn) * 8],
                                num_idxs=pn * P,
                                num_idxs_reg=pn * P,
                                elem_size=D,
                                single_packet=SINGLE_PACKET,
                                queue_num=b % NQUEUES,
                            )
                for tl in range(G):
                    t = g * G + tl
                    ps_mean = ps_mean_pool.tile([P, P], f32)
                    for s in range(KT):
                        # slot s -> (bank, chunk-in-seg)
                        b = int(np.searchsorted(slot_off[t], s, side="right")) - 1
                        j = s - int(slot_off[t][b])
                        cpos = int(chunk_off[t][b] - chunk_off[g * G][b]) + j
                        oh = oh_pool.tile([P, P], bf16)
                        nc.vector.tensor_scalar(
                            oh[:],
                            iota_g[:],
                            dst_all[:, t, s : s + 1],
                            w_all[:, t, s : s + 1],
                            mybir.AluOpType.is_equal,
                            mybir.AluOpType.mult,
                        )
                        nc.tensor.matmul(
                            out=ps_mean[:],
                            lhsT=msg[:, b, cpos, :],
                            rhs=oh[:],
                            start=(s == 0),
                            stop=(s == KT - 1),
                        )
                    meanT_sb = ep_pool.tile([P, P], f32, tag="meanT")
                    nc.scalar.copy(meanT_sb[:], ps_mean[:])

                    ps_o = ps_out_pool.tile([P, P], f32)
                    nc.tensor.matmul(
                        out=ps_o[:], lhsT=meanT_sb[:], rhs=wl_sb[:],
                        start=True, stop=False,
                    )
                    nc.tensor.matmul(
                        out=ps_o[:],
                        lhsT=xT_all[:, t * P : (t + 1) * P],
                        rhs=wr_sb[:],
                        start=False, stop=False,
                    )
                    nc.tensor.matmul(
                        out=ps_o[:], lhsT=ones1[:], rhs=bl_sb[:],
                        start=False, stop=True,
                    )
                    nc.vector.tensor_copy(obuf[:, t, :], ps_o[:])
                    sq_scr = ep_pool.tile([P, P], f32, tag="sq")
                    nc.scalar.activation(
                        sq_scr[:],
                        ps_o[:],
                        mybir.ActivationFunctionType.Square,
                        accum_out=nbuf[:, t : t + 1],
                    )

            # ---- epilogue: row-wise L2 normalize, store ----
            nrm = ep_pool.tile([P, TILES_PER_CORE], f32, tag="nrm")
            nc.scalar.sqrt(nrm[:], nbuf[:])
            nrmc = ep_pool.tile([P, TILES_PER_CORE], f32, tag="nrmc")
            nc.vector.tensor_scalar_max(nrmc[:], nrm[:], 1e-12)
            rn = ep_pool.tile([P, TILES_PER_CORE], f32, tag="rn")
            nc.vector.reciprocal(rn[:], nrmc[:])
            for t in range(TILES_PER_CORE):
                nc.vector.tensor_scalar(
                    obuf[:, t, :],
                    obuf[:, t, :],
                    rn[:, t : t + 1],
                    None,
                    mybir.AluOpType.mult,
                )
                nc.sync.dma_start(out[t * P : (t + 1) * P, :], obuf[:, t, :])
            rep_ctx.__exit__(None, None, None)

    nc.compile()
    return nc


def _pack_core(vb, kb):
    """Greedy node->tile packing under per-(tile,bank) edge capacities.

    vb: [n_nodes, NBANKS] per-node per-bank in-degree. Returns assign
    [n_nodes] tile ids, or None if infeasible.
    """
    n = vb.shape[0]
    cap = kb * P  # [T, B] edge capacity
    rem = cap.astype(np.int64).copy()
    slots = np.full(TILES_PER_CORE, P, np.int64)
    assign = np.full(n, -1, np.int64)
    order = np.argsort(-vb.sum(1), kind="stable")
    for node in order:
        v = vb[node]
        ok = (slots > 0) & (rem >= v[None, :]).all(1)
        if not ok.any():
            return None
        score = (rem - v[None, :]).min(1)
        score[~ok] = -(1 << 40)
        t = int(np.argmax(score))
        assign[node] = t
        rem[t] -= v
        slots[t] -= 1
    return assign


def _prepare(edge_index, force_template=None):
    """Host-side sharding: shard by dst range, pack tiles, build gather meta."""
    src = np.ascontiguousarray(edge_index[0]).astype(np.int64)
    dst = np.ascontiguousarray(edge_index[1]).astype(np.int64)

    cnt = np.bincount(dst, minlength=N_NODES)
    w_node = (1.0 / np.maximum(cnt, 1)).astype(np.float32)

    order = np.argsort(dst, kind="stable")
    src_s = src[order]
    dst_s = dst[order]

    core_edges = []
    for c in range(N_CORES):
        base = c * NODES_PER_CORE
        lo = np.searchsorted(dst_s, base)
        hi = np.searchsorted(dst_s, base + NODES_PER_CORE)
        core_edges.append((src_s[lo:hi], dst_s[lo:hi] - base))

    # try templates cheapest-first; all cores must pack under the same one
    templates = (
        (force_template,)
        if force_template
        else ("seg5444", "5444", "5544", "5555")
    )
    for template in templates:
        kb = _kbmat(template)
        assigns = []
        for c in range(N_CORES):
            s_c, d_c = core_edges[c]
            vb = np.zeros((NODES_PER_CORE, NBANKS), np.int64)
            np.add.at(vb, (d_c, s_c // BANK), 1)
            a = _pack_core(vb, kb)
            if a is None:
                break
            assigns.append(a)
        if len(assigns) == N_CORES:
            break
    KT, slot_off, chunk_off, CB, OFFB, segs, MW = _derive(kb)
    TOTCH = int(CB.sum())

    gidx = np.zeros((N_CORES, P, TOTCH * 8), np.int16)
    dstrel = np.full((N_CORES, P, TILES_PER_CORE, KT), -1.0, np.float32)
    wgt = np.zeros((N_CORES, P, TILES_PER_CORE, KT), np.float32)
    perm = np.full((N_CORES, NODE_PAD), -1, np.int64)  # slot -> local node id

    prow = np.arange(P) % 16
    for c in range(N_CORES):
        s_c, d_c = core_edges[c]
        a = assigns[c]
        # position of each node within its tile (by assignment order)
        tile_order = np.argsort(a, kind="stable")
        pos = np.zeros(NODES_PER_CORE, np.int64)
        tcounts = np.bincount(a, minlength=TILES_PER_CORE)
        starts = np.concatenate([[0], np.cumsum(tcounts)])
        for t in range(TILES_PER_CORE):
            nodes_t = tile_order[starts[t] : starts[t + 1]]
            pos[nodes_t] = np.arange(len(nodes_t))
            perm[c, t * P : t * P + len(nodes_t)] = nodes_t
        # edges -> (tile, bank) groups
        et = a[d_c]
        eb = s_c // BANK
        key = et * NBANKS + eb
        ordc = np.argsort(key, kind="stable")
        s_e, d_e = s_c[ordc], d_c[ordc]
        counts = np.bincount(key[ordc], minlength=TILES_PER_CORE * NBANKS)
        gstarts = np.concatenate([[0], np.cumsum(counts)])
        for t in range(TILES_PER_CORE):
            for b in range(NBANKS):
                gkey = t * NBANKS + b
                n = counts[gkey]
                nslots = kb[t, b] * P
                assert n <= nslots, (t, b, n, nslots)
                lo = gstarts[gkey]
                sv = s_e[lo : lo + n] - b * BANK
                dv = pos[d_e[lo : lo + n]].astype(np.float32)
                wv = w_node[d_e[lo : lo + n] + c * NODES_PER_CORE]
                i_pad = np.zeros(nslots, np.int16)
                i_pad[:n] = sv.astype(np.int16)
                d_pad = np.full(nslots, -1.0, np.float32)
                d_pad[:n] = dv
                w_pad = np.zeros(nslots, np.float32)
                w_pad[:n] = wv
                # bank-stream columns for this group's chunks
                col0 = (OFFB[b] + chunk_off[t, b]) * 8
                ncol = kb[t, b] * 8
                scol = np.arange(ncol) * 16
                gidx[c, :, col0 : col0 + ncol] = i_pad[
                    scol[None, :] + prow[:, None]
                ]
                s0 = slot_off[t][b]
                dstrel[c, :, t, s0 : s0 + kb[t, b]] = d_pad.reshape(-1, P).T
                wgt[c, :, t, s0 : s0 + kb[t, b]] = w_pad.reshape(-1, P).T

    return gidx, dstrel, wgt, perm, template


def kernel(x, edge_index, W_l, b_l, W_r):
    import ml_dtypes
    from concourse.bass_utils import run_bass_kernel_spmd

    x = np.ascontiguousarray(np.asarray(x, dtype=np.float32))
    W_l = np.ascontiguousarray(np.asarray(W_l, dtype=np.float32))
    W_r = np.ascontiguousarray(np.asarray(W_r, dtype=np.float32))
    b_l = np.ascontiguousarray(np.asarray(b_l, dtype=np.float32)).reshape(1, D)

    gidx, dstrel, wgt, perm, template = _prepare(np.asarray(edge_index))

    xpad = np.zeros((X_PAD_ROWS, D), np.float32)
    xpad[:N_NODES] = x
    xbf = xpad.astype(ml_dtypes.bfloat16)
    iota = np.broadcast_to(
        np.arange(P, dtype=np.float32)[None, :], (P, P)
    ).astype(ml_dtypes.bfloat16)

    if template not in _program_cache:
        _program_cache[template] = _build_program(template)
    nc = _program_cache[template]

    in_maps = []
    for c in range(N_CORES):
        base = c * NODES_PER_CORE
        pc = perm[c]
        xperm = np.zeros((NODE_PAD, D), np.float32)
        valid = pc >= 0
        xperm[valid] = x[base + pc[valid]]
        m = {
            "xbf": xbf,
            "xT": np.ascontiguousarray(xperm.T),
            "gidx": gidx[c],
            "dstrel": dstrel[c],
            "wgt": wgt[c],
            "iota": np.ascontiguousarray(iota),
            "wl": W_l,
            "wr": W_r,
            "bl": b_l,
        }
        in_maps.append(m)

    LAST["nc"] = nc
    LAST["in_maps"] = in_maps
    LAST["perm"] = perm
    r = run_bass_kernel_spmd(nc, in_maps, list(range(N_CORES)), trace=TRACE)
    LAST["exec_time_ns"] = r.exec_time_ns
    res = r.results
    return unpermute(perm, [res[c]["out"] for c in range(N_CORES)])


def unpermute(perm, raw):
    """Map device output rows (tile-slot order) back to node order."""
    out = np.empty((N_NODES, D), np.float32)
    for c in range(N_CORES):
        base = c * NODES_PER_CORE
        pc = perm[c]
        valid = pc >= 0
        out[base + pc[valid]] = raw[c][valid]
    return out


# revision 26
# speedup vs baseline: 78.2474x; 1.0370x over previous
"""BipartiteSAGEConv on 8 Trainium2 NeuronCores.

out = normalize(mean_{dst}(x[src]) @ W_l + b_l + x @ W_r)

Strategy (v3):
- Host: shard destination nodes across the 8 cores (each core owns 12500
  contiguous nodes and all edges pointing into them -> no cross-core
  reduction). Within a core, nodes are PACKED into 98 tiles of 128 by a
  greedy bin-packer so that each (tile, src-bank) edge group fits a fixed
  rotating chunk template KB[t][b] = 5 if b == t%4 else 4 (17 chunks/tile,
  213k gather indices vs 251k for the uniform 5,5,5,5 template). The
  per-edge weight w = 1/max(deg(dst),1) is folded into the bf16 one-hot so
  PSUM accumulation yields the mean directly.
- Device (SPMD, identical program on all 8 cores):
  * SWDGE dma_gather of x[src] rows (bf16, 256B each) from 4 HBM banks of
    25024 rows (int16 index limit). Descriptor generation on the GPSIMD Q7
    cores is the kernel's critical path (~4ns/idx, fully serial), so total
    index count is what matters. Calls are capped at 7 chunks = 896 idx
    (57 descs/DMA, under the fixed 64-desc SWDGE ring; more hangs the HW)
    and rotate across 4 queues so transfers drain between same-queue calls.
    Indices stream from HBM per 7-tile segment (double-buffered).
  * DVE builds the weighted one-hot (iota == dstrel) * w; PE accumulates
    meanT[f, n] += msg[e, f].T @ onehot[e, n] over the tile's 17 chunks,
    then one PSUM group: out = meanT.T @ W_l + xT.T @ W_r + ones x b_l,
    with xT pre-transposed on the host.
  * Results collect in one SBUF buffer [128, 98, 128]; row sum-of-squares
    per tile via ACT Square+accum; sqrt/clamp/reciprocal once on [128, 98];
    final scale per tile; output rows are un-permuted on the host.
"""

import numpy as np

N_NODES = 100000
D = 128
P = 128
N_CORES = 8
NODES_PER_CORE = N_NODES // N_CORES  # 12500
TILES_PER_CORE = (NODES_PER_CORE + P - 1) // P  # 98
NODE_PAD = TILES_PER_CORE * P  # 12544
X_PAD_ROWS = 100096  # 782 * 128
BANK = X_PAD_ROWS // 4  # 25024 rows per gather bank (< 32768 int16 limit)
NBANKS = 4
G = 7  # tiles per segment (98 = 14 * 7)
NBATCH = TILES_PER_CORE // G  # 14
MAXCH = 7  # max chunks per dma_gather call (896 idx = 57 descs <= 64 ring)

NQUEUES = 4
SCRATCH = 16384
SINGLE_PACKET = False

_program_cache = {}

# test harness hooks
TRACE = False
LAST = {}


def _kbmat(template):
    """Per-(tile, bank) chunk counts for a template id."""
    kb = np.zeros((TILES_PER_CORE, NBANKS), np.int64)
    for t in range(TILES_PER_CORE):
        for b in range(NBANKS):
            if template == "seg5444":
                # fat bank fixed per segment -> (seg, bank) chunk counts are
                # 35 or 28, both of which split into fewer max-7-chunk calls
                kb[t, b] = 5 if b == (t // G) % 4 else 4
            elif template == "5444":
                kb[t, b] = 5 if b == t % 4 else 4
            elif template == "5544":
                kb[t, b] = 5 if b in (t % 4, (t + 1) % 4) else 4
            else:  # "5555"
                kb[t, b] = 5
    return kb


def _derive(kb):
    """Static layout bookkeeping from the chunk-count matrix."""
    KT = int(kb[0].sum())  # same for every tile by construction
    assert (kb.sum(1) == KT).all()
    slot_off = np.concatenate(
        [np.zeros((TILES_PER_CORE, 1), np.int64), np.cumsum(kb, 1)], 1
    )  # [T, 5]
    chunk_off = np.concatenate(
        [np.zeros((1, NBANKS), np.int64), np.cumsum(kb, 0)], 0
    )  # [T+1, B] per-bank chunk prefix over tiles
    CB = chunk_off[-1]  # chunks per bank
    OFFB = np.concatenate([[0], np.cumsum(CB)])  # bank col-block offsets (chunks)
    # per segment: chunk ranges and call partition
    segs = []
    MW = 0
    for g in range(NBATCH):
        per_bank = []
        for b in range(NBANKS):
            c0 = int(chunk_off[g * G, b])
            c1 = int(chunk_off[(g + 1) * G, b])
            cg = c1 - c0
            MW = max(MW, cg)
            npieces = -(-cg // MAXCH)
            base, extra = divmod(cg, npieces)
            sizes = [base + (1 if i < extra else 0) for i in range(npieces)]
            per_bank.append((c0, cg, sizes))
        segs.append(per_bank)
    return KT, slot_off, chunk_off, CB, OFFB, segs, MW


def _build_program(template, ablate="", bench_repeat=1, tail="tile"):
    """Build + compile the SPMD Bass program for a chunk template."""
    ablate_set = set(ablate.split(",")) if ablate else set()
    import contextlib

    import concourse.bass as bass
    import concourse.tile as tile
    from concourse import bacc, mybir

    f32 = mybir.dt.float32
    bf16 = mybir.dt.bfloat16
    i16 = mybir.dt.int16

    kb = _kbmat(template)
    KT, slot_off, chunk_off, CB, OFFB, segs, MW = _derive(kb)
    TOTCH = int(CB.sum())

    nc = bacc.Bacc(
        "TRN2",
        target_bir_lowering=False,
        debug=False,
        num_devices=N_CORES,
        num_swdge_queues=NQUEUES,
        dynamic_dma_scratch_size=SCRATCH,
    )

    xbf = nc.dram_tensor("xbf", [X_PAD_ROWS, D], bf16, kind="ExternalInput")
    xT_in = nc.dram_tensor("xT", [D, NODE_PAD], f32, kind="ExternalInput")
    gidx = nc.dram_tensor("gidx", [P, TOTCH * 8], i16, kind="ExternalInput")
    dstrel = nc.dram_tensor("dstrel", [P, TILES_PER_CORE, KT], f32, kind="ExternalInput")
    wgt = nc.dram_tensor("wgt", [P, TILES_PER_CORE, KT], f32, kind="ExternalInput")
    iota_in = nc.dram_tensor("iota", [P, P], bf16, kind="ExternalInput")
    wl = nc.dram_tensor("wl", [D, D], f32, kind="ExternalInput")
    wr = nc.dram_tensor("wr", [D, D], f32, kind="ExternalInput")
    bl = nc.dram_tensor("bl", [1, D], f32, kind="ExternalInput")
    out = nc.dram_tensor("out", [NODE_PAD, D], f32, kind="ExternalOutput")

    with tile.TileContext(nc) as tc:
        with (
            tc.tile_pool(name="const", bufs=1) as const_pool,
            tc.tile_pool(name="meta", bufs=1) as meta_pool,
            tc.tile_pool(name="idx", bufs=2) as idx_pool,
            tc.tile_pool(name="msg", bufs=2) as msg_pool,
            tc.tile_pool(name="oh", bufs=6) as oh_pool,
            tc.tile_pool(name="ep", bufs=3) as ep_pool,
            tc.tile_pool(name="ps_mean", bufs=2, space="PSUM") as ps_mean_pool,
            tc.tile_pool(name="ps_out", bufs=3, space="PSUM") as ps_out_pool,
        ):
            # ---- constants / weights / metadata (loaded once) ----
            iota_g = const_pool.tile([P, P], bf16)
            nc.sync.dma_start(iota_g[:], iota_in[:])
            wl_sb = const_pool.tile([D, D], f32)
            nc.sync.dma_start(wl_sb[:], wl[:])
            wr_sb = const_pool.tile([D, D], f32)
            nc.sync.dma_start(wr_sb[:], wr[:])
            bl_sb = const_pool.tile([1, D], f32)
            nc.sync.dma_start(bl_sb[:], bl[:])
            ones1 = const_pool.tile([1, D], f32)
            nc.vector.memset(ones1[:], 1.0)

            xT_all = const_pool.tile([D, NODE_PAD], f32)
            nc.sync.dma_start(xT_all[:], xT_in[:])
            dst_all = meta_pool.tile([P, TILES_PER_CORE, KT], f32)
            nc.sync.dma_start(dst_all[:], dstrel[:])
            w_all = meta_pool.tile([P, TILES_PER_CORE, KT], f32)
            nc.sync.dma_start(w_all[:], wgt[:])

            if tail != "tile":
                obuf = const_pool.tile([P, TILES_PER_CORE, P], f32)
                nbuf = const_pool.tile([P, TILES_PER_CORE], f32)

            idxbs = [
                idx_pool.tile([P, NBANKS, MW * 8], i16, tag="idx", name=f"idxb{i}")
                for i in range(2)
            ]

            def load_idx(g, buf):
                for b in range(NBANKS):
                    c0, cg, _ = segs[g][b]
                    nc.sync.dma_start(
                        buf[:, b, : cg * 8],
                        gidx[:, (OFFB[b] + c0) * 8 : (OFFB[b] + c0 + cg) * 8],
                    )

            load_idx(0, idxbs[0])

            if "gather" in ablate_set:
                msg_fixed = [
                    msg_pool.tile(
                        [P, NBANKS, MW, D], bf16, tag="msg", name=f"msgf{i}"
                    )
                    for i in range(2)
                ]
                for mt in msg_fixed:
                    nc.vector.memset(mt[:], 0.01)

            rep_ctx = (
                tc.For_i(0, bench_repeat, 1)
                if bench_repeat > 1
                else contextlib.nullcontext()
            )
            rep_ctx.__enter__()
            for g in range(NBATCH):
                if g + 1 < NBATCH:
                    load_idx(g + 1, idxbs[(g + 1) % 2])
                if "gather" in ablate_set:
                    msg = msg_fixed[g % 2]
                else:
                    msg = msg_pool.tile([P, NBANKS, MW, D], bf16, tag="msg")
                    # emit calls round-robin across banks (= queues) so
                    # same-queue calls are ~4 desc-gen slots apart and each
                    # transfer drains before its queue's ring is reused
                    maxp = max(len(segs[g][b][2]) for b in range(NBANKS))
                    for pi in range(maxp):
                        for b in range(NBANKS):
                            c0, cg, sizes = segs[g][b]
                            if pi >= len(sizes):
                                continue
                            p0 = sum(sizes[:pi])
                            pn = sizes[pi]
                            nc.gpsimd.dma_gather(
                                out_ap=msg[:, b, p0 : p0 + pn, :],
                                in_ap=xbf[b * BANK : (b + 1) * BANK, :],
                                idxs_ap=idxbs[g % 2][:, b, p0 * 8 : (p0 + pn) * 8],
                                num_idxs=pn * P,
                                num_idxs_reg=pn * P,
                                elem_size=D,
                                single_packet=SINGLE_PACKET,
                                queue_num=b % NQUEUES,
                            )
                for tl in range(G):
                    t = g * G + tl
                    ps_mean = ps_mean_pool.tile([P, P], f32)
                    for s in range(KT):
                        # slot s -> (bank, chunk-in-seg)
                        b = int(np.searchsorted(slot_off[t], s, side="right")) - 1
                        j = s - int(slot_off[t][b])
                        cpos = int(chunk_off[t][b] - chunk_off[g * G][b]) + j
                        oh = oh_pool.tile([P, P], bf16)
                        nc.vector.tensor_scalar(
                            oh[:],
                            iota_g[:],
                            dst_all[:, t, s : s + 1],
                            w_all[:, t, s : s + 1],
                            mybir.AluOpType.is_equal,
                            mybir.AluOpType.mult,
                        )
                        nc.tensor.matmul(
                            out=ps_mean[:],
                            lhsT=msg[:, b, cpos, :],
                            rhs=oh[:],
                            start=(s == 0),
                            stop=(s == KT - 1),
                        )
                    meanT_sb = ep_pool.tile([P, P], f32, tag="meanT")
                    nc.scalar.copy(meanT_sb[:], ps_mean[:])

                    ps_o = ps_out_pool.tile([P, P], f32)
                    nc.tensor.matmul(
                        out=ps_o[:], lhsT=meanT_sb[:], rhs=wl_sb[:],
                        start=True, stop=False,
                    )
                    nc.tensor.matmul(
                        out=ps_o[:],
                        lhsT=xT_all[:, t * P : (t + 1) * P],
                        rhs=wr_sb[:],
                        start=False, stop=False,
                    )
                    nc.tensor.matmul(
                        out=ps_o[:], lhsT=ones1[:], rhs=bl_sb[:],
                        start=False, stop=True,
                    )
                    if tail == "tile":
                        # per-tile L2 normalize + store: overlaps under the
                        # gathers instead of a serial epilogue after them
                        sq_scr = ep_pool.tile([P, P], f32, tag="sq")
                        ss = ep_pool.tile([P, 1], f32, tag="ss")
                        nc.scalar.activation(
                            sq_scr[:],
                            ps_o[:],
                            mybir.ActivationFunctionType.Square,
                            accum_out=ss[:],
                        )
                        nrm = ep_pool.tile([P, 1], f32, tag="nrm")
                        nc.scalar.sqrt(nrm[:], ss[:])
                        nrmc = ep_pool.tile([P, 1], f32, tag="nrmc")
                        nc.vector.tensor_scalar_max(nrmc[:], nrm[:], 1e-12)
                        rn = ep_pool.tile([P, 1], f32, tag="rn")
                        nc.vector.reciprocal(rn[:], nrmc[:])
                        outt = ep_pool.tile([P, P], f32, tag="outt")
                        nc.vector.tensor_scalar(
                            outt[:],
                            ps_o[:],
                            rn[:, :1],
                            None,
                            mybir.AluOpType.mult,
                        )
                        nc.sync.dma_start(out[t * P : (t + 1) * P, :], outt[:])
                    else:
                        nc.vector.tensor_copy(obuf[:, t, :], ps_o[:])
                        sq_scr = ep_pool.tile([P, P], f32, tag="sq")
                        nc.scalar.activation(
                            sq_scr[:],
                            ps_o[:],
                            mybir.ActivationFunctionType.Square,
                            accum_out=nbuf[:, t : t + 1],
                        )

            if tail != "tile":
                # ---- epilogue: row-wise L2 normalize, store ----
                nrm = ep_pool.tile([P, TILES_PER_CORE], f32, tag="nrm")
                nc.scalar.sqrt(nrm[:], nbuf[:])
                nrmc = ep_pool.tile([P, TILES_PER_CORE], f32, tag="nrmc")
                nc.vector.tensor_scalar_max(nrmc[:], nrm[:], 1e-12)
                rn = ep_pool.tile([P, TILES_PER_CORE], f32, tag="rn")
                nc.vector.reciprocal(rn[:], nrmc[:])
                for t in range(TILES_PER_CORE):
                    nc.vector.tensor_scalar(
                        obuf[:, t, :],
                        obuf[:, t, :],
                        rn[:, t : t + 1],
                        None,
                        mybir.AluOpType.mult,
                    )
                    nc.sync.dma_start(out[t * P : (t + 1) * P, :], obuf[:, t, :])
            rep_ctx.__exit__(None, None, None)

    nc.compile()
    return nc


def _pack_core(vb, kb):
    """Greedy node->tile packing under per-(tile,bank) edge capacities.

    vb: [n_nodes, NBANKS] per-node per-bank in-degree. Returns assign
    [n_nodes] tile ids, or None if infeasible.
    """
    n = vb.shape[0]
    cap = kb * P  # [T, B] edge capacity
    rem = cap.astype(np.int64).copy()
    slots = np.full(TILES_PER_CORE, P, np.int64)
    assign = np.full(n, -1, np.int64)
    order = np.argsort(-vb.sum(1), kind="stable")
    for node in order:
        v = vb[node]
        ok = (slots > 0) & (rem >= v[None, :]).all(1)
        if not ok.any():
            return None
        score = (rem - v[None, :]).min(1)
        score[~ok] = -(1 << 40)
        t = int(np.argmax(score))
        assign[node] = t
        rem[t] -= v
        slots[t] -= 1
    return assign


def _prepare(edge_index, force_template=None):
    """Host-side sharding: shard by dst range, pack tiles, build gather meta."""
    src = np.ascontiguousarray(edge_index[0]).astype(np.int64)
    dst = np.ascontiguousarray(edge_index[1]).astype(np.int64)

    cnt = np.bincount(dst, minlength=N_NODES)
    w_node = (1.0 / np.maximum(cnt, 1)).astype(np.float32)

    order = np.argsort(dst, kind="stable")
    src_s = src[order]
    dst_s = dst[order]

    core_edges = []
    for c in range(N_CORES):
        base = c * NODES_PER_CORE
        lo = np.searchsorted(dst_s, base)
        hi = np.searchsorted(dst_s, base + NODES_PER_CORE)
        core_edges.append((src_s[lo:hi], dst_s[lo:hi] - base))

    # try templates cheapest-first; all cores must pack under the same one
    templates = (
        (force_template,)
        if force_template
        else ("seg5444", "5444", "5544", "5555")
    )
    for template in templates:
        kb = _kbmat(template)
        assigns = []
        for c in range(N_CORES):
            s_c, d_c = core_edges[c]
            vb = np.zeros((NODES_PER_CORE, NBANKS), np.int64)
            np.add.at(vb, (d_c, s_c // BANK), 1)
            a = _pack_core(vb, kb)
            if a is None:
                break
            assigns.append(a)
        if len(assigns) == N_CORES:
            break
    KT, slot_off, chunk_off, CB, OFFB, segs, MW = _derive(kb)
    TOTCH = int(CB.sum())

    gidx = np.zeros((N_CORES, P, TOTCH * 8), np.int16)
    dstrel = np.full((N_CORES, P, TILES_PER_CORE, KT), -1.0, np.float32)
    wgt = np.zeros((N_CORES, P, TILES_PER_CORE, KT), np.float32)
    perm = np.full((N_CORES, NODE_PAD), -1, np.int64)  # slot -> local node id

    prow = np.arange(P) % 16
    for c in range(N_CORES):
        s_c, d_c = core_edges[c]
        a = assigns[c]
        # position of each node within its tile (by assignment order)
        tile_order = np.argsort(a, kind="stable")
        pos = np.zeros(NODES_PER_CORE, np.int64)
        tcounts = np.bincount(a, minlength=TILES_PER_CORE)
        starts = np.concatenate([[0], np.cumsum(tcounts)])
        for t in range(TILES_PER_CORE):
            nodes_t = tile_order[starts[t] : starts[t + 1]]
            pos[nodes_t] = np.arange(len(nodes_t))
            perm[c, t * P : t * P + len(nodes_t)] = nodes_t
        # edges -> (tile, bank) groups
        et = a[d_c]
        eb = s_c // BANK
        key = et * NBANKS + eb
        ordc = np.argsort(key, kind="stable")
        s_e, d_e = s_c[ordc], d_c[ordc]
        counts = np.bincount(key[ordc], minlength=TILES_PER_CORE * NBANKS)
        gstarts = np.concatenate([[0], np.cumsum(counts)])
        for t in range(TILES_PER_CORE):
            for b in range(NBANKS):
                gkey = t * NBANKS + b
                n = counts[gkey]
                nslots = kb[t, b] * P
                assert n <= nslots, (t, b, n, nslots)
                lo = gstarts[gkey]
                sv = s_e[lo : lo + n] - b * BANK
                dv = pos[d_e[lo : lo + n]].astype(np.float32)
                wv = w_node[d_e[lo : lo + n] + c * NODES_PER_CORE]
                i_pad = np.zeros(nslots, np.int16)
                i_pad[:n] = sv.astype(np.int16)
                d_pad = np.full(nslots, -1.0, np.float32)
                d_pad[:n] = dv
                w_pad = np.zeros(nslots, np.float32)
                w_pad[:n] = wv
                # bank-stream columns for this group's chunks
                col0 = (OFFB[b] + chunk_off[t, b]) * 8
                ncol = kb[t, b] * 8
                scol = np.arange(ncol) * 16
                gidx[c, :, col0 : col0 + ncol] = i_pad[
                    scol[None, :] + prow[:, None]
                ]
                s0 = slot_off[t][b]
                dstrel[c, :, t, s0 : s0 + kb[t, b]] = d_pad.reshape(-1, P).T
                wgt[c, :, t, s0 : s0 + kb[t, b]] = w_pad.reshape(-1, P).T

    return gidx, dstrel, wgt, perm, template


def kernel(x, edge_index, W_l, b_l, W_r):
    import ml_dtypes
    from concourse.bass_utils import run_bass_kernel_spmd

    x = np.ascontiguousarray(np.asarray(x, dtype=np.float32))
    W_l = np.ascontiguousarray(np.asarray(W_l, dtype=np.float32))
    W_r = np.ascontiguousarray(np.asarray(W_r, dtype=np.float32))
    b_l = np.ascontiguousarray(np.asarray(b_l, dtype=np.float32)).reshape(1, D)

    gidx, dstrel, wgt, perm, template = _prepare(np.asarray(edge_index))

    xpad = np.zeros((X_PAD_ROWS, D), np.float32)
    xpad[:N_NODES] = x
    xbf = xpad.astype(ml_dtypes.bfloat16)
    iota = np.broadcast_to(
        np.arange(P, dtype=np.float32)[None, :], (P, P)
    ).astype(ml_dtypes.bfloat16)

    if template not in _program_cache:
        _program_cache[template] = _build_program(template)
    nc = _program_cache[template]

    in_maps = []
    for c in range(N_CORES):
        base = c * NODES_PER_CORE
        pc = perm[c]
        xperm = np.zeros((NODE_PAD, D), np.float32)
        valid = pc >= 0
        xperm[valid] = x[base + pc[valid]]
        m = {
            "xbf": xbf,
            "xT": np.ascontiguousarray(xperm.T),
            "gidx": gidx[c],
            "dstrel": dstrel[c],
            "wgt": wgt[c],
            "iota": np.ascontiguousarray(iota),
            "wl": W_l,
            "wr": W_r,
            "bl": b_l,
        }
        in_maps.append(m)

    LAST["nc"] = nc
    LAST["in_maps"] = in_maps
    LAST["perm"] = perm
    r = run_bass_kernel_spmd(nc, in_maps, list(range(N_CORES)), trace=TRACE)
    LAST["exec_time_ns"] = r.exec_time_ns
    res = r.results
    return unpermute(perm, [res[c]["out"] for c in range(N_CORES)])


def unpermute(perm, raw):
    """Map device output rows (tile-slot order) back to node order."""
    out = np.empty((N_NODES, D), np.float32)
    for c in range(N_CORES):
        base = c * NODES_PER_CORE
        pc = perm[c]
        valid = pc >= 0
        out[base + pc[valid]] = raw[c][valid]
    return out
